# revision 1
# baseline (speedup 1.0000x reference)
"""Causal single-head attention (shared-weight multi-head), 8-core Trainium2 Bass kernel.

V3: streamed input DMA, paired exp (ACT on [128,2,512] PSUM tiles), PV in
transposed orientation (V1 stationary -> o^T [65, 512], one PSUM bank per
slot), unnormalized output + denominator shipped to host (divide+transpose
on host), projections interleaved between attention pairs so the tensor
engine's background weight loads hide and ACT never starves.

Sharding: 8 cores = 4 batches x 2 roles; role 0 owns blocks {0,3,4,7}, role 1
{1,2,5,6} of 8x512 rows; host permutes blocks so owned sit at positions 0-3.
Per-slot other-region padded to PADS[s] blocks, validity via padmask data.
"""

import os
import numpy as np
import ml_dtypes

B, T, E, HEAD, NH = 4, 4096, 1024, 64, 16
BLK = 512
NCORES = 8
OWN = {0: [0, 3, 4, 7], 1: [1, 2, 5, 6]}
PADS = [1, 2, 3, 4]
PADMASK = {0: [0.0, 1.0, 0.0, 1.0], 1: [1.0, 0.0, 1.0, 0.0]}

_prog_cache = {}


def _build_program(reps=None):
    import concourse.bass as bass
    import concourse.mybir as mybir
    import concourse.tile as tile
    from concourse import bacc

    f32 = mybir.dt.float32
    bf16 = mybir.dt.bfloat16

    nc = bacc.Bacc("TRN2", target_bir_lowering=False, debug=False, num_devices=NCORES)

    xT = nc.dram_tensor("xT", [E, T], bf16, kind="ExternalInput").ap()
    wpack = nc.dram_tensor("wpack", [128, E // 128, 320], bf16,
                           kind="ExternalInput").ap()
    tri = nc.dram_tensor("tri", [128, 896], bf16, kind="ExternalInput").ap()
    padmask = nc.dram_tensor("padmask", [128, 4], f32, kind="ExternalInput").ap()
    # unnormalized head^T plus denominator row, divided on host
    out = nc.dram_tensor("out", [HEAD + 1, 4, BLK], f32, kind="ExternalOutput").ap()

    KE = E // 128

    import contextlib

    with tile.TileContext(nc) as tc:
        loop_ctx = tc.For_i(0, reps, 1) if reps else contextlib.nullcontext()
        with (
            loop_ctx,
            tc.tile_pool(name="singles", bufs=1) as singles,
            tc.tile_pool(name="psum_proj", bufs=2, space="PSUM") as psum_proj,
            tc.tile_pool(name="psum_s", bufs=2, space="PSUM") as psum_s,
            tc.tile_pool(name="psum_o", bufs=2, space="PSUM") as psum_o,
            tc.tile_pool(name="ptil", bufs=6) as ptil_pool,
        ):
            # ---- streamed input DMAs ----
            wp_sb = singles.tile([128, KE, 320], bf16)
            nc.sync.dma_start(out=wp_sb, in_=wpack)
            wk2_sb = wp_sb[:, :, 0:128]
            wq2_sb = wp_sb[:, :, 128:256]
            wv_sb = wp_sb[:, :, 256:320]
            pm_sb = singles.tile([128, 4], f32)

            x_sb = singles.tile([128, KE, T], bf16)

            def xdma(b, klo=0, khi=KE):
                nc.sync.dma_start(
                    out=x_sb[:, klo:khi, b * BLK:(b + 1) * BLK],
                    in_=xT[klo * 128:khi * 128, b * BLK:(b + 1) * BLK].rearrange(
                        "(k p) t -> p k t", p=128
                    ),
                )

            tri_sb = singles.tile([128, 896], bf16)
            xdma(0, 0, 4)
            xdma(0, 4, 8)
            nc.sync.dma_start(out=pm_sb, in_=padmask)
            xdma(4, 0, 4)
            xdma(4, 4, 8)
            nc.sync.dma_start(out=tri_sb, in_=tri)
            for b in (1, 5, 2, 6, 3, 7):
                xdma(b)

            # ---- PE warmup: keep the tensor engine busy during the initial
            # DMA window so the p-state ramp reaches full clock in time ----
            warm = singles.tile([1, BLK], bf16)
            nc.gpsimd.memset(warm, 0.0)
            wps = psum_proj.tile([1, BLK], f32, tag="proj", name="warm")
            for i in range(5):
                nc.tensor.matmul(wps, warm[:, 0:1], warm,
                                 start=(i == 0), stop=(i == 4))

            KTb = [singles.tile([128, BLK], bf16, name=f"ktb{b}")
                   for b in range(8)]
            QTb = [singles.tile([128, BLK], bf16, name=f"qtb{b}")
                   for b in range(4)]
            V1b = [singles.tile([128, 4, HEAD + 1], bf16, name=f"v1b{b}")
                   for b in range(8)]

            def kproj_unit(blk, wsb, dst):
                def emit():
                    ps = psum_proj.tile([128, BLK], f32, tag="proj")
                    for k in range(KE):
                        nc.tensor.matmul(
                            ps, wsb[:, k, :],
                            x_sb[:, k, blk * BLK:(blk + 1) * BLK],
                            start=(k == 0), stop=(k == KE - 1),
                        )
                    nc.vector.tensor_copy(dst, ps)
                return emit

            def vproj_unit(blk, c):
                def emit():
                    if c == 0:
                        nc.vector.memset(V1b[blk][:, :, HEAD:HEAD + 1], 1.0)
                    g = blk * 4 + c
                    ps = psum_proj.tile([128, HEAD], f32, tag="proj")
                    for k in range(KE):
                        nc.tensor.matmul(
                            ps, x_sb[:, k, g * 128:(g + 1) * 128], wv_sb[:, k, :],
                            start=(k == 0), stop=(k == KE - 1),
                        )
                    nc.vector.tensor_copy(V1b[blk][:, c, 0:HEAD], ps)
                return emit

            def proj_units(blk, with_q):
                u = [kproj_unit(blk, wk2_sb, KTb[blk])]
                if with_q:
                    u.append(kproj_unit(blk, wq2_sb, QTb[blk]))
                u += [vproj_unit(blk, c) for c in range(4)]
                return u

            outs_sb = singles.tile([HEAD + 1, 4, BLK], f32)

            for s in range(4):
                bx = 4 + s
                own_chunks = 4 * (s + 1)
                other_chunks = 4 * PADS[s]
                nP = (own_chunks + other_chunks) // 2
                o_ps = psum_o.tile([HEAD + 1, BLK], f32, tag="o",
                                   name=f"o_s{s}")

                def attn_pair(pi, g0, s=s, own_chunks=own_chunks, nP=nP,
                              o_ps=o_ps):
                    # pair of consecutive global chunks (same 512-block)
                    s_ps = psum_s.tile([128, 2, BLK], f32, tag="s")
                    subs = []
                    for j in range(2):
                        g = g0 + j
                        blk = g // 4 if g < 16 else 4 + (g - 16) // 4
                        sub = g % 4
                        subs.append((blk, sub))
                        lo, hi = (0, 64) if j == 0 else (64, 128)
                        nc.tensor.matmul(
                            s_ps[:, j, :],
                            KTb[blk][lo:hi, sub * 128:(sub + 1) * 128],
                            QTb[s][lo:hi, :], start=True, stop=True,
                        )
                    pt = ptil_pool.tile([128, 2, BLK], bf16, tag="pt")
                    nc.scalar.activation(
                        pt, s_ps, mybir.ActivationFunctionType.Exp, scale=0.125
                    )
                    ci0 = pi * 2
                    for j in range(2):
                        ci = ci0 + j
                        if ci >= 4 * s and ci < own_chunks:
                            d = ci - 4 * s
                            nc.vector.tensor_mul(
                                pt[:, j, :], pt[:, j, :],
                                tri_sb[:, 384 - 128 * d:896 - 128 * d]
                            )
                    if ci0 >= own_chunks and \
                            (ci0 - own_chunks) // 4 == PADS[s] - 1:
                        nc.vector.tensor_scalar_mul(pt, pt, pm_sb[:, s:s + 1])
                    for j in range(2):
                        blk, sub = subs[j]
                        nc.tensor.matmul(
                            o_ps, V1b[blk][:, sub, :], pt[:, j, :],
                            start=(pi == 0 and j == 0),
                            stop=(pi == nP - 1 and j == 1),
                        )

                def emit_interleaved(pair_list, units):
                    # spread proj units between attention pairs
                    nu, npair = len(units), len(pair_list)
                    ui = 0
                    for idx, (pi, g0) in enumerate(pair_list):
                        attn_pair(pi, g0)
                        want = (idx + 1) * nu // max(npair, 1)
                        while ui < want:
                            units[ui]()
                            ui += 1
                    while ui < nu:
                        units[ui]()
                        ui += 1

                if s == 0:
                    for u in proj_units(0, True):
                        u()
                own_pairs = [(p, 2 * p) for p in range(own_chunks // 2)]
                oth_pairs = [
                    (own_chunks // 2 + p, 16 + 2 * p)
                    for p in range(other_chunks // 2)
                ]
                for u in proj_units(bx, False):
                    u()
                emit_interleaved(own_pairs, [])
                next_units = proj_units(s + 1, True) if s < 3 else []
                emit_interleaved(oth_pairs, next_units)

                nc.vector.tensor_copy(outs_sb[:, s, :], o_ps)
                nc.sync.dma_start(out=out[:, s, :], in_=outs_sb[:, s, :])

    nc.compile()
    return nc


def _host_inputs(embedded, Wq, Wk, Wv):
    bf = ml_dtypes.bfloat16
    emb = np.asarray(embedded, dtype=np.float32)
    wq = np.asarray(Wq, dtype=np.float32).astype(bf)
    wk = np.asarray(Wk, dtype=np.float32).astype(bf)
    wv = np.asarray(Wv, dtype=np.float32).astype(bf)
    # packed, k-chunked weights: [128, KE, 320] with duplicated wk/wq halves
    KE = E // 128
    wpack = np.empty((128, KE, 320), np.float32)
    for k in range(KE):
        wpack[:, k, 0:64] = wk[k * 128:(k + 1) * 128]
        wpack[:, k, 64:128] = wk[k * 128:(k + 1) * 128]
        wpack[:, k, 128:192] = wq[k * 128:(k + 1) * 128]
        wpack[:, k, 192:256] = wq[k * 128:(k + 1) * 128]
        wpack[:, k, 256:320] = wv[k * 128:(k + 1) * 128]
    wpack = wpack.astype(bf)

    # master mask: chunk d's tri slice is tri[:, 384-128d : 896-128d],
    # i.e. T[p, u] = (p <= u - 384)
    p = np.arange(128)[:, None]
    u = np.arange(896)[None, :]
    tri = (p <= u - 384).astype(bf)

    in_maps = []
    for b in range(B):
        for role in range(2):
            order = OWN[role] + OWN[1 - role]
            xTb = emb[b].T
            xTp = np.concatenate(
                [xTb[:, j * BLK:(j + 1) * BLK] for j in order], axis=1
            ).astype(bf)
            pm = np.broadcast_to(
                np.asarray(PADMASK[role], np.float32), (128, 4)
            ).astype(np.float32)
            in_maps.append({
                "xT": np.ascontiguousarray(xTp),
                "wpack": np.ascontiguousarray(wpack),
                "tri": np.ascontiguousarray(tri),
                "padmask": np.ascontiguousarray(pm),
            })
    return in_maps


def _run(nc, in_maps, trace=False):
    from concourse.bass_utils import run_bass_kernel_spmd
    return run_bass_kernel_spmd(nc, in_maps, list(range(NCORES)), trace=trace)


def _assemble(results):
    head = np.empty((B, T, HEAD), dtype=np.float32)
    for core, r in enumerate(results):
        b, role = divmod(core, 2)
        o = np.asarray(r["out"])  # [65, 4, 512] unnormalized head^T + denom
        vals = o[:HEAD] / o[HEAD:HEAD + 1]  # [64, 4, 512]
        for s in range(4):
            j = OWN[role][s]
            head[b, j * BLK:(j + 1) * BLK, :] = vals[:, s, :].T
    return np.tile(head, (1, 1, NH))


def kernel(embedded, Wq, Wk, Wv, num_heads):
    num_heads = int(num_heads)
    assert num_heads == NH

    if "nc" not in _prog_cache:
        _prog_cache["nc"] = _build_program()
    nc = _prog_cache["nc"]

    in_maps = _host_inputs(embedded, Wq, Wk, Wv)
    res = _run(nc, in_maps, trace=bool(int(os.environ.get("KERNEL_TRACE", "0"))))
    _prog_cache["last_result"] = res
    return _assemble(res.results)

